# revision 2
# baseline (speedup 1.0000x reference)
"""Sparse attention (RoPE'd Q=K, strictly-causal unnormalized scores @ V).

  Q: (1, 4, 2048, 8192) f32   V: (1, 1, 2048, 256) f32
  out = tril(QR @ QR^T, -1) @ V   per head, V broadcast over heads.

Sharding: 8 cores = 4 heads x 2 halves of the N=8192 contraction dim.
Each core computes a full (2048, 256) partial output from its
(2048, 4096) slice of Q; host sums the two halves per head.

v2: host work and axon upload minimized. Q is shipped once per core in
natural [t, n] layout as float16 (16 MB/core vs 64 MB of f32 packed
layouts in v1). RoPE runs on device (phases -> magic-round frac ->
Sin activation; cos via add_range_wrap), the [n, t] operand layout is
produced by on-device DMA transposes, and all matmuls run in fp16 with
f32 PSUM accumulation.

Device algorithm (chunked linear attention, chunk C=256):
  out[t] = QR[t] @ S_{<chunk} + (intra-chunk causal part), where
  S = sum_s QR[s] (x) V[s] is an [N_c, D] state accumulated chunk by chunk.
"""

import math

import numpy as np

THETA = 2.0**16
TWO_PI = 2.0 * math.pi
MAGIC = 1.5 * 2.0**23  # float add trick: round-to-nearest-int for |x|<2^22

B, NH, T, N, D = 1, 4, 2048, 8192, 256
NSPLIT = 2
NCORES = NH * NSPLIT
P = 128
NC_FEAT = N // NSPLIT  # 4096 features per core
JW = NC_FEAT // 2  # 2048 rotary pairs per core
KT = NC_FEAT // P  # 32 n-tiles
TT = T // P  # 16 t-tiles
C = 256  # chunk length
NCH = T // C  # 8 chunks
CSUB = C // P  # 2 t-subtiles per chunk

_COMPILED = None


def _build():
    import concourse.tile as tile
    import concourse.alu_op_type as alu
    from concourse import bacc, mybir

    nc = bacc.Bacc(
        "TRN2",
        target_bir_lowering=False,
        debug=False,
        enable_asserts=False,
        num_devices=NCORES,
    )
    f32 = mybir.dt.float32
    f16 = mybir.dt.float16
    i32 = mybir.dt.int32
    AF = mybir.ActivationFunctionType
    ADD, MUL = alu.AluOpType.add, alu.AluOpType.mult

    q = nc.dram_tensor("q", [T, NC_FEAT], f16, kind="ExternalInput").ap()
    v = nc.dram_tensor("v", [P, TT * D], f16, kind="ExternalInput").ap()
    fr = nc.dram_tensor("fr", [1, JW], f32, kind="ExternalInput").ap()
    out = nc.dram_tensor("out", [T, D], f32, kind="ExternalOutput").ap()

    W = JW // 2  # table/rope piece width (pairs)

    with tile.TileContext(nc) as tc:
        with (
            tc.tile_pool(name="cst", bufs=1) as cp,
            tc.tile_pool(name="qp", bufs=3) as qp,
            tc.tile_pool(name="tb32", bufs=2) as t32p,
            tc.tile_pool(name="tb16", bufs=2) as t16p,
            tc.tile_pool(name="qr", bufs=3) as qrp,
            tc.tile_pool(name="qrt", bufs=2) as qtp,
            tc.tile_pool(name="st", bufs=KT) as stp,
            tc.tile_pool(name="sc", bufs=4) as sp,
            tc.tile_pool(name="ob", bufs=3) as op_,
            tc.tile_pool(name="pi", bufs=2, space="PSUM") as ppi,
            tc.tile_pool(name="po", bufs=2, space="PSUM") as ppo,
            tc.tile_pool(name="pu", bufs=3, space="PSUM") as ppu,
        ):
            # ---- one-time setup ----
            with tc.high_priority():
                f1 = cp.tile([1, JW], f32)
                nc.scalar.dma_start(out=f1, in_=fr)
                vt = cp.tile([P, TT * D], f16)
                nc.scalar.dma_start(out=vt, in_=v)
            vtiles = [vt[:, a * D : (a + 1) * D] for a in range(TT)]
            fb = cp.tile([P, JW], f32)
            nc.gpsimd.partition_broadcast(fb, f1)

            tcol_i = cp.tile([P, 1], i32)
            nc.gpsimd.iota(tcol_i, pattern=[[0, 1]], channel_multiplier=1)
            tcol0 = cp.tile([P, 1], f32)
            nc.vector.tensor_copy(tcol0, tcol_i)
            halfpi = cp.tile([P, 1], f32)
            nc.vector.memset(halfpi, math.pi / 2)

            # masks generated on device: mask[i][p, j] = 1.0 iff p + 128*i < j
            jrow_i = cp.tile([P, C], i32)
            nc.gpsimd.iota(jrow_i, pattern=[[1, C]], channel_multiplier=0)
            jrow = cp.tile([P, C], f32)
            nc.vector.tensor_copy(jrow, jrow_i)
            mtiles = []
            for i in range(CSUB):
                pcol = cp.tile([P, 1], f32, tag=f"pc{i}")
                nc.vector.tensor_scalar_add(pcol, tcol0, float(P * i))
                mt = cp.tile([P, C], f32, tag=f"mt{i}")
                nc.vector.tensor_scalar(
                    mt, jrow, pcol, None, alu.AluOpType.is_gt
                )
                mtiles.append(mt)

            # force setup loads to land before any chunk work
            dum = cp.tile([P, 2], f32)
            nc.vector.tensor_copy(dum[:, 0:1], vt[:, 0:1])
            nc.vector.tensor_copy(dum[:, 1:2], fb[:, 0:1])

            Stiles = [
                stp.tile([P, D], f16, tag="S", name=f"S{k}") for k in range(KT)
            ]

            for c in range(NCH):
                c0 = c * C
                # ---- per t-subtile: load q, build tables, rope, transpose ----
                qrt = qtp.tile([P, KT * C], f16, tag="qrt", name=f"qrt{c}")
                qr_m = []
                for m in range(CSUB):
                    t_idx = CSUB * c + m
                    qt = qp.tile([P, NC_FEAT], f16, tag="q", name=f"q{c}_{m}")
                    nc.sync.dma_start(
                        out=qt, in_=q[t_idx * P : (t_idx + 1) * P, :]
                    )

                    tcol = t32p.tile([P, 1], f32, tag="tc")
                    nc.vector.tensor_scalar_add(tcol, tcol0, float(t_idx * P))

                    qrt_m = qrp.tile([P, NC_FEAT], f16, tag="qr", name=f"qr{c}_{m}")
                    qe_f = qt.rearrange("p (j two) -> p j two", two=2)[:, :, 0]
                    qo_f = qt.rearrange("p (j two) -> p j two", two=2)[:, :, 1]
                    re_f = qrt_m.rearrange("p (j two) -> p j two", two=2)[:, :, 0]
                    ro_f = qrt_m.rearrange("p (j two) -> p j two", two=2)[:, :, 1]

                    for w in range(JW // W):
                        jsl = slice(w * W, (w + 1) * W)
                        ph = t32p.tile([P, W], f32, tag="ph")
                        nc.vector.tensor_scalar_mul(ph, fb[:, jsl], tcol)
                        rnd = t32p.tile([P, W], f32, tag="rnd")
                        nc.vector.tensor_scalar(rnd, ph, MAGIC, -MAGIC, ADD, ADD)
                        frac = t32p.tile([P, W], f32, tag="frac")
                        nc.vector.scalar_tensor_tensor(frac, rnd, -1.0, ph, MUL, ADD)
                        sin_t = t16p.tile([P, W], f16, tag="sin")
                        nc.scalar.activation(sin_t, frac, AF.Sin, scale=TWO_PI)
                        aa = t32p.tile([P, W], f32, tag="aa")
                        nc.scalar.activation(aa, frac, AF.Abs, scale=TWO_PI)
                        cos_t = t16p.tile([P, W], f16, tag="cos")
                        nc.scalar.activation(
                            cos_t, aa, AF.Sin, scale=-1.0, bias=halfpi
                        )

                        qe, qo = qe_f[:, jsl], qo_f[:, jsl]
                        re, ro = re_f[:, jsl], ro_f[:, jsl]
                        tmp = t16p.tile([P, W], f16, tag="tmp")
                        tmp2 = t16p.tile([P, W], f16, tag="tmp2")
                        nc.vector.tensor_mul(tmp, qo, sin_t)
                        nc.vector.tensor_mul(tmp2, qe, cos_t)
                        nc.vector.tensor_sub(re, tmp2, tmp)
                        nc.vector.tensor_mul(tmp, qo, cos_t)
                        nc.vector.tensor_mul(tmp2, qe, sin_t)
                        nc.vector.tensor_add(ro, tmp, tmp2)
                    qr_m.append(qrt_m)

                    for k in range(KT):
                        nc.scalar.dma_start_transpose(
                            out=qrt[:, k * C + m * P : k * C + (m + 1) * P],
                            in_=qrt_m[:, k * P : (k + 1) * P],
                        )

                def qslice(k, lo, hi):
                    return qrt[:, k * C + lo : k * C + hi]

                # ---- intra-chunk causal scores, [s, t] upper layout ----
                st_c = []
                for a in range(CSUB):
                    ps = ppi.tile([P, C], f32)
                    for k in range(KT):
                        nc.tensor.matmul(
                            ps,
                            lhsT=qslice(k, a * P, a * P + P),
                            rhs=qslice(k, 0, C),
                            start=(k == 0),
                            stop=(k == KT - 1),
                        )
                    st = sp.tile([P, C], f16)
                    nc.vector.tensor_mul(st, ps, mtiles[a])
                    st_c.append(st)

                # ---- out rows of this chunk: q @ S_{<c} + intra @ V ----
                ot = op_.tile([P, CSUB * D], f32)
                for m in range(CSUB):
                    po = ppo.tile([P, D], f32)
                    first = True
                    if c > 0:
                        for k in range(KT):
                            nc.tensor.matmul(
                                po,
                                lhsT=qslice(k, m * P, m * P + P),
                                rhs=Stiles[k],
                                start=first,
                                stop=False,
                            )
                            first = False
                    for a in range(m + 1):
                        nc.tensor.matmul(
                            po,
                            lhsT=st_c[a][:, m * P : (m + 1) * P],
                            rhs=vtiles[CSUB * c + a],
                            start=first,
                            stop=(a == m),
                        )
                        first = False
                    nc.vector.tensor_copy(ot[:, m * D : (m + 1) * D], po)
                out_rows = out[c0 : c0 + C, :].rearrange("(m p) d -> p m d", p=P)
                nc.sync.dma_start(
                    out=out_rows, in_=ot.rearrange("p (m d) -> p m d", m=CSUB)
                )

                # ---- state update: S[k] += qr_chunk[:, k-tile].T @ V_chunk ----
                if c == NCH - 1:
                    continue
                for k in range(KT):
                    pu = ppu.tile([P, D], f32)
                    for m in range(CSUB):
                        nc.tensor.matmul(
                            pu,
                            lhsT=qr_m[m][:, k * P : (k + 1) * P],
                            rhs=vtiles[CSUB * c + m],
                            start=(m == 0),
                            stop=(m == CSUB - 1),
                        )
                    if c == 0:
                        nc.vector.tensor_copy(Stiles[k], pu)
                    else:
                        nc.vector.tensor_add(Stiles[k], Stiles[k], pu)

    nc.compile()
    return nc


def _get_compiled():
    global _COMPILED
    if _COMPILED is None:
        _COMPILED = _build()
    return _COMPILED


def _masks_host():
    si = np.arange(P)[:, None]
    tj = np.arange(C)[None, :]
    return np.concatenate(
        [(si + P * i < tj).astype(np.float32) for i in range(CSUB)], axis=0
    )  # [256, 256]


def _freqs_host(half):
    jg = (half * JW + np.arange(JW)).astype(np.float64)
    f = 1.0 / (np.float32(THETA) ** ((2.0 * jg) / np.float64(N))) / TWO_PI
    return f.astype(np.float32)[None, :]


def kernel(Q, V, _want_results=False, **_unused):
    from concourse import bass_utils

    Q = np.asarray(Q)
    V = np.asarray(V)

    v16 = np.ascontiguousarray(
        V[0, 0].reshape(TT, P, D).transpose(1, 0, 2).reshape(P, TT * D)
    ).astype(np.float16)
    frs = [_freqs_host(half) for half in range(NSPLIT)]

    in_maps = []
    for h in range(NH):
        for half in range(NSPLIT):
            q16 = np.ascontiguousarray(
                Q[0, h, :, half * NC_FEAT : (half + 1) * NC_FEAT],
                dtype=np.float16,
            )
            in_maps.append({"q": q16, "v": v16, "fr": frs[half]})

    nc = _get_compiled()
    res = bass_utils.run_bass_kernel_spmd(nc, in_maps, core_ids=list(range(NCORES)))

    out = np.empty((B, NH, T, D), dtype=np.float32)
    for h in range(NH):
        out[0, h] = res.results[2 * h]["out"] + res.results[2 * h + 1]["out"]
    if _want_results:
        return out, res
    return out


if __name__ == "__main__":
    rng = np.random.default_rng(0)
    Q = (rng.standard_normal((B, NH, T, N)) * 0.02).astype(np.float32)
    V = rng.standard_normal((B, 1, T, D)).astype(np.float32)
    out = kernel(Q=Q, V=V)
    print("out", out.shape, out.dtype, float(np.abs(out).max()))
